# revision 1
# baseline (speedup 1.0000x reference)
"""LoRA QKV fused projection kernel for 8 TRN2 NeuronCores.

Reference computation (T=8192 tokens, HID=4096, D=6144 out, S=8 slots, R=16):
    y = x @ W.T
    a[t,s,i,r] = sum_h x[t,h] * lora_A[s,i,r,h]         (down-proj, all slots)
    a *= onehot(token_to_slot)[t,s] * scaling[s]         (routing gate)
    d[t, :] = concat_i( sum_{s,r} a[t,s,i,r] * B_i[s,:,r] )   (up-proj)
    out = y + d

Sharding (chosen for minimal LDWEIGHTS serialization on the PE):
  * main GEMM: K-split — core c takes hidden dims [c*512, (c+1)*512) and
    computes a full [6144, 8192] fp32 partial of y^T; host reduces the 8
    partials. W k-slice (12.6 MB) streams once; x k-slice (16.8 MB) stays
    resident in SBUF as the moving operand, so each W 128x128 tile is loaded
    into the PE array exactly once per 4 N=512 matmuls.
  * LoRA path: token-split — core c handles tokens [c*1024, (c+1)*1024) with
    the full hidden dim: aT = A @ x_shard^T (PSUM-accumulated over all 32
    k-tiles), multiplied by a host-built gate matrix (onehot * scaling,
    expanded over ranks), then up-projected with B as the stationary operand.
  * all matmuls run in float32r (single-pass fp32 mode, ~1.5e-4 rel err,
    4x the throughput of classic fp32 which needs 2 half-rate passes).

Host pre/post: pure layout rearranges + the 8-way fp32 partial reduce and
final transpose. All routing logic (gate) is exact fp32.
"""

import numpy as np

# problem shape (hardcoded per harness contract)
T = 8192
HID = 4096
Q_SIZE = 4096
KV_SIZE = 1024
D = Q_SIZE + 2 * KV_SIZE  # 6144
S = 8
R = 16
NCORES = 8
P = 128

KC = HID // NCORES        # 512 hidden dims per core (main GEMM K-shard)
KT = KC // P              # 4 k-tiles per core
TC = T // NCORES          # 1024 tokens per core (LoRA shard)
MB = D // P               # 48 output row-blocks of 128
KA = HID // P             # 32 k-tiles for the LoRA down-proj
NT = T // 512             # 16 moving n-tiles of 512 tokens
NG = 4                    # psum groups of 4 tiles (4 banks, double buffered)
NJ = NT // NG             # 4 n-tiles per psum group

_CACHE = {}


def _build_nc():
    import concourse.mybir as mybir
    import concourse.tile as tile
    from concourse import bacc

    dtr = mybir.dt.float32r
    f32 = mybir.dt.float32

    nc = bacc.Bacc(None, target_bir_lowering=False, debug=False)

    # ---- DRAM parameters (per-core shapes; declaration order = binding order)
    x_res_d = nc.declare_dram_parameter("x_res", [P, KT, T], dtr, isOutput=False)
    w_d = nc.declare_dram_parameter("w_t", [MB, P, KT, P], dtr, isOutput=False)
    xl_d = nc.declare_dram_parameter("x_lora", [P, KA, TC], dtr, isOutput=False)
    a_d = nc.declare_dram_parameter("a_t", [P, KA, 3, P], dtr, isOutput=False)
    b_d = nc.declare_dram_parameter("b_t", [P, MB, P], dtr, isOutput=False)
    g_d = nc.declare_dram_parameter("gate", [P, TC], f32, isOutput=False)
    y_d = nc.declare_dram_parameter("y_part", [MB, P, T], f32, isOutput=True)
    d_d = nc.declare_dram_parameter("d_out", [MB, P, TC], f32, isOutput=True)

    with tile.TileContext(nc) as tc:
        with tc.tile_pool(name="xres", bufs=1) as xres_pool, \
             tc.tile_pool(name="big", bufs=3) as big_pool, \
             tc.tile_pool(name="wp", bufs=3) as w_pool, \
             tc.tile_pool(name="ap", bufs=2) as a_pool, \
             tc.tile_pool(name="agp", bufs=1) as ag_pool, \
             tc.tile_pool(name="bp", bufs=3) as b_pool, \
             tc.tile_pool(name="dsp", bufs=3) as ds_pool, \
             tc.tile_pool(name="psum", bufs=8, space="PSUM") as ps_pool:

            # resident moving operand for the main GEMM: [p(k), kt, t]
            x_res = xres_pool.tile([P, KT, T], dtr, tag="xres")
            nc.sync.dma_start(out=x_res[:], in_=x_res_d[:])

            # ---------------- Phase A: LoRA down-proj aT = A @ xl ----------
            # aT[(i,sr), t] accumulated in 6 psum tiles over all 32 k-tiles.
            ps_a = [
                ps_pool.tile([P, 512], f32, tag="ps", name=f"ps_a{i}_{h}")
                for i in range(3) for h in range(2)
            ]
            KCH = 2  # k-tiles per streamed chunk
            for ch in range(KA // KCH):
                xl_t = big_pool.tile([P, KCH, TC], dtr, tag="big", name=f"xl{ch}")
                nc.sync.dma_start(out=xl_t[:], in_=xl_d[:, ch * KCH:(ch + 1) * KCH, :])
                a_t = a_pool.tile([P, KCH, 3, P], dtr, tag="a", name=f"a{ch}")
                nc.sync.dma_start(out=a_t[:], in_=a_d[:, ch * KCH:(ch + 1) * KCH, :, :])
                for kk in range(KCH):
                    first = ch == 0 and kk == 0
                    last = ch == KA // KCH - 1 and kk == KCH - 1
                    for i in range(3):
                        for h in range(2):
                            nc.tensor.matmul(
                                ps_a[i * 2 + h][:],
                                a_t[:, kk, i, :],
                                xl_t[:, kk, h * 512:(h + 1) * 512],
                                start=first, stop=last,
                            )

            # ---------------- Phase B: routing gate ------------------------
            gate_t = ag_pool.tile([P, TC], f32, tag="gate")
            nc.sync.dma_start(out=gate_t[:], in_=g_d[:])
            ag = []
            for i in range(3):
                ag_t = ag_pool.tile([P, TC], dtr, tag=f"ag{i}", name=f"ag{i}")
                for h in range(2):
                    sl = slice(h * 512, (h + 1) * 512)
                    nc.vector.tensor_mul(ag_t[:, sl], ps_a[i * 2 + h][:], gate_t[:, sl])
                ag.append(ag_t)

            # ---------------- Phase C: main GEMM partials -------------------
            # out_part[(mb,dl), t] += W_kc[:, mb].T @ x_kc  for this core's
            # hidden k-shard.  W tile is stationary: 1 LDWEIGHTS per (mb, ng,
            # kt) amortized over NJ=4 N=512 matmuls.
            for mb in range(MB):
                w_t = w_pool.tile([P, KT, P], dtr, tag="w", name=f"w{mb}")
                nc.sync.dma_start(out=w_t[:], in_=w_d[mb])
                for ng in range(NG):
                    pss = [
                        ps_pool.tile([P, 512], f32, tag="ps", name=f"pm{mb}_{ng}_{j}")
                        for j in range(NJ)
                    ]
                    for kk in range(KT):
                        for j in range(NJ):
                            t0 = (ng * NJ + j) * 512
                            nc.tensor.matmul(
                                pss[j][:],
                                w_t[:, kk, :],
                                x_res[:, kk, t0:t0 + 512],
                                start=(kk == 0), stop=(kk == KT - 1),
                            )
                    st = big_pool.tile([P, NJ * 512], f32, tag="big", name=f"st{mb}_{ng}")
                    for j in range(NJ):
                        nc.vector.tensor_copy(st[:, j * 512:(j + 1) * 512], pss[j][:])
                    nc.scalar.dma_start(
                        out=y_d[mb, :, ng * NJ * 512:(ng + 1) * NJ * 512], in_=st[:]
                    )

            # ---------------- Phase D: LoRA up-proj -------------------------
            import concourse.mybir as _mybir
            for mb in range(MB):
                b_t = b_pool.tile([P, P], dtr, tag="b", name=f"b{mb}")
                nc.sync.dma_start(out=b_t[:], in_=b_d[:, mb, :])
                i = 0 if mb < Q_SIZE // P else (1 if mb < (Q_SIZE + KV_SIZE) // P else 2)
                for h in range(2):
                    ps_u = ps_pool.tile([P, 512], f32, tag="ps", name=f"pu{mb}_{h}")
                    nc.tensor.matmul(
                        ps_u[:],
                        b_t[:],
                        ag[i][:, h * 512:(h + 1) * 512],
                        start=True, stop=True,
                    )
                    dst = ds_pool.tile([P, 512], f32, tag="dst", name=f"d{mb}_{h}")
                    nc.vector.tensor_copy(dst[:], ps_u[:])
                    nc.scalar.dma_start(out=d_d[mb, :, h * 512:(h + 1) * 512], in_=dst[:])

    nc.compile()
    return nc


def _get_nc():
    if "nc" not in _CACHE:
        _CACHE["nc"] = _build_nc()
    return _CACHE["nc"]


def _prep_in_maps(x, W, lora_A, lora_B_q, lora_B_k, lora_B_v, scaling, token_to_slot):
    f = np.float32
    x = np.ascontiguousarray(x, dtype=f)
    W = np.ascontiguousarray(W, dtype=f)

    # main GEMM moving operand: [c, p, kt, t]  (h = c*512 + kt*128 + p)
    x_res = np.ascontiguousarray(
        x.reshape(T, NCORES, KT, P).transpose(1, 3, 2, 0))
    # main GEMM stationary: [c, mb, p, kt, dl]  (d = mb*128 + dl)
    w_t = np.ascontiguousarray(
        W.reshape(MB, P, NCORES, KT, P).transpose(2, 0, 4, 3, 1))
    # LoRA down-proj moving operand: [c, p, ka, tl]  (t = c*1024 + tl)
    x_lora = np.ascontiguousarray(
        x.reshape(NCORES, TC, KA, P).transpose(0, 3, 2, 1))
    # LoRA A stationary: [p, ka, i, (s r)]
    a_t = np.ascontiguousarray(
        np.asarray(lora_A, dtype=f).reshape(S, 3, R, KA, P).transpose(4, 3, 1, 0, 2)
        .reshape(P, KA, 3, S * R))
    # LoRA B stationary: [(s r), mb, dl]
    bq = np.asarray(lora_B_q, dtype=f).transpose(0, 2, 1).reshape(S * R, Q_SIZE)
    bk = np.asarray(lora_B_k, dtype=f).transpose(0, 2, 1).reshape(S * R, KV_SIZE)
    bv = np.asarray(lora_B_v, dtype=f).transpose(0, 2, 1).reshape(S * R, KV_SIZE)
    b_t = np.ascontiguousarray(
        np.concatenate([bq, bk, bv], axis=1).reshape(S * R, MB, P))
    # routing gate, expanded over ranks: [c, (s r), tl]
    slot = np.asarray(token_to_slot).reshape(NCORES, TC)
    g = (slot[:, None, :] == np.arange(S, dtype=slot.dtype)[None, :, None])
    g = g.astype(f) * np.asarray(scaling, dtype=f)[None, :, None]
    gate = np.ascontiguousarray(np.repeat(g, R, axis=1))

    in_maps = []
    for c in range(NCORES):
        in_maps.append({
            "x_res": x_res[c],
            "w_t": w_t[c],
            "x_lora": x_lora[c],
            "a_t": a_t,
            "b_t": b_t,
            "gate": gate[c],
        })
    return in_maps


def _assemble(results):
    # reduce the 8 main-GEMM partials: [mb, dl, t] fp32
    acc = results[0]["y_part"].copy()
    for c in range(1, NCORES):
        acc += results[c]["y_part"]
    yT = acc.reshape(D, T)
    # add each core's LoRA delta into its token columns
    for c in range(NCORES):
        yT[:, c * TC:(c + 1) * TC] += results[c]["d_out"].reshape(D, TC)
    return np.ascontiguousarray(yT.T)


def _run(inputs, trace=False):
    from concourse.bass_utils import run_bass_kernel_spmd
    nc = _get_nc()
    in_maps = _prep_in_maps(**inputs)
    res = run_bass_kernel_spmd(
        nc, in_maps, core_ids=list(range(NCORES)), trace=trace)
    return res


def kernel(**inputs) -> np.ndarray:
    res = _run(inputs, trace=False)
    return _assemble(res.results)


if __name__ == "__main__":
    rng = np.random.default_rng(0)
    ins = {
        "x": rng.standard_normal((T, HID)).astype(np.float32),
        "W": (rng.standard_normal((D, HID)) * 0.02).astype(np.float32),
        "lora_A": (rng.standard_normal((S, 3, R, HID)) * 0.02).astype(np.float32),
        "lora_B_q": (rng.standard_normal((S, Q_SIZE, R)) * 0.02).astype(np.float32),
        "lora_B_k": (rng.standard_normal((S, KV_SIZE, R)) * 0.02).astype(np.float32),
        "lora_B_v": (rng.standard_normal((S, KV_SIZE, R)) * 0.02).astype(np.float32),
        "scaling": rng.uniform(0.5, 2.0, S).astype(np.float32),
        "token_to_slot": rng.integers(0, S, T).astype(np.int32),
    }
    out = kernel(**ins)
    print("out", out.shape, out.dtype)



# revision 2
# speedup vs baseline: 1.1399x; 1.1399x over previous
"""LoRA QKV fused projection kernel for 8 TRN2 NeuronCores.

Reference computation (T=8192 tokens, HID=4096, D=6144 out, S=8 slots, R=16):
    y = x @ W.T
    a[t,s,i,r] = sum_h x[t,h] * lora_A[s,i,r,h]          (down-proj, all slots)
    a *= onehot(token_to_slot)[t,s] * scaling[s]          (routing gate)
    d[t, :] = concat_i( sum_{s,r} a[t,s,i,r] * B_i[s,:,r] )   (up-proj)
    out = y + d

Sharding: 2D grid, 2-way over output rows x 4-way over tokens.
  core c -> (dh = c//4, tq = c%4): tokens [tq*2048, (tq+1)*2048) and an
  interleaved half of the output rows (half of Q + half of K + half of V
  rows, so the q/k/v block pattern is identical on every core and the
  SPMD program needs no per-core branching).

All matmul operands are bf16 (fp32 PSUM accumulate): 2x the PE rate of
fp32r, rel err ~2e-3 vs the 2e-2 gate.  Per core:
  * Phase A (LoRA down-proj): aT[i] = A_i @ x_shard^T accumulated over
    32 k-tiles, gated (onehot*scaling, host-built) into bf16 ag tiles.
  * Main loop over 24 row-blocks of 128: y-tile accumulated over 32
    k-tiles with W stationary (1 LDWEIGHTS per 4 N=512 matmuls), then
    ONE extra matmul accumulates the LoRA delta (B stationary, ag
    moving) into the same PSUM bank before the fp32->bf16 store.
Host: pure layout rearranges; final assembly is a cast + row-permuted
transpose per block (no reduce).
"""

import numpy as np
import ml_dtypes

# problem shape (hardcoded per harness contract)
T = 8192
HID = 4096
Q_SIZE = 4096
KV_SIZE = 1024
D = Q_SIZE + 2 * KV_SIZE  # 6144
S = 8
R = 16
NCORES = 8
P = 128

DS = 2                    # output-dim shards
TS = 4                    # token shards
TC = T // TS              # 2048 tokens per core
DH = D // DS              # 3072 output rows per core
MB = DH // P              # 24 row-blocks of 128
KA = HID // P             # 32 k-tiles
NJ = TC // 512            # 4 moving n-tiles of 512 tokens
# q/k/v target per row-block (same on every core thanks to row interleave)
I_OF_MB = [0] * (Q_SIZE // DS // P) + [1] * (KV_SIZE // DS // P) + [2] * (KV_SIZE // DS // P)

BF16 = np.dtype(ml_dtypes.bfloat16)

_CACHE = {}


def _rows_of_dh(dh):
    """Global output-row indices owned by output-shard dh (q+k+v halves)."""
    return np.concatenate([
        np.arange(dh * Q_SIZE // DS, (dh + 1) * Q_SIZE // DS),
        np.arange(Q_SIZE + dh * KV_SIZE // DS, Q_SIZE + (dh + 1) * KV_SIZE // DS),
        np.arange(Q_SIZE + KV_SIZE + dh * KV_SIZE // DS,
                  Q_SIZE + KV_SIZE + (dh + 1) * KV_SIZE // DS),
    ])


def _build_nc():
    import concourse.mybir as mybir
    import concourse.tile as tile
    from concourse import bacc

    bf16 = mybir.dt.bfloat16
    f32 = mybir.dt.float32

    nc = bacc.Bacc(None, target_bir_lowering=False, debug=False)

    x_d = nc.declare_dram_parameter("x_sh", [P, KA, TC], bf16, isOutput=False)
    w_d = nc.declare_dram_parameter("w_sh", [MB, P, KA, P], bf16, isOutput=False)
    a_d = nc.declare_dram_parameter("a_sh", [P, KA, 3, P], bf16, isOutput=False)
    b_d = nc.declare_dram_parameter("b_sh", [P, MB, P], bf16, isOutput=False)
    g_d = nc.declare_dram_parameter("gate", [P, TC], f32, isOutput=False)
    y_d = nc.declare_dram_parameter("y_out", [MB, P, TC], bf16, isOutput=True)

    with tile.TileContext(nc) as tc:
        with tc.tile_pool(name="xres", bufs=1) as xres_pool, \
             tc.tile_pool(name="wp", bufs=2) as w_pool, \
             tc.tile_pool(name="cst", bufs=1) as cst_pool, \
             tc.tile_pool(name="agp", bufs=1) as ag_pool, \
             tc.tile_pool(name="stp", bufs=2) as st_pool, \
             tc.tile_pool(name="psum", bufs=8, space="PSUM") as ps_pool:

            # resident moving operand: [p(k), kt, t] for both GEMM paths
            x_res = xres_pool.tile([P, KA, TC], bf16, tag="xres")
            nc.sync.dma_start(out=x_res[:], in_=x_d[:])
            a_res = cst_pool.tile([P, KA, 3, P], bf16, tag="a")
            nc.sync.dma_start(out=a_res[:], in_=a_d[:])
            b_res = cst_pool.tile([P, MB, P], bf16, tag="b")
            nc.sync.dma_start(out=b_res[:], in_=b_d[:])
            gate_t = cst_pool.tile([P, TC], f32, tag="gate")
            nc.sync.dma_start(out=gate_t[:], in_=g_d[:])

            # ---------------- Phase A: LoRA down-proj + gate ----------------
            ag = []
            for i in range(3):
                pss = [
                    ps_pool.tile([P, 512], f32, tag="ps", name=f"pa{i}_{j}")
                    for j in range(NJ)
                ]
                for kk in range(KA):
                    for j in range(NJ):
                        nc.tensor.matmul(
                            pss[j][:],
                            a_res[:, kk, i, :],
                            x_res[:, kk, j * 512:(j + 1) * 512],
                            start=(kk == 0), stop=(kk == KA - 1),
                        )
                ag_t = ag_pool.tile([P, TC], bf16, tag=f"ag{i}", name=f"ag{i}")
                for j in range(NJ):
                    sl = slice(j * 512, (j + 1) * 512)
                    nc.vector.tensor_mul(ag_t[:, sl], pss[j][:], gate_t[:, sl])
                ag.append(ag_t)

            # ------------- Main GEMM + fused LoRA up-proj -------------------
            for mb in range(MB):
                w_t = w_pool.tile([P, KA, P], bf16, tag="w", name=f"w{mb}")
                nc.sync.dma_start(out=w_t[:], in_=w_d[mb])
                pss = [
                    ps_pool.tile([P, 512], f32, tag="ps", name=f"pm{mb}_{j}")
                    for j in range(NJ)
                ]
                for kk in range(KA):
                    for j in range(NJ):
                        nc.tensor.matmul(
                            pss[j][:],
                            w_t[:, kk, :],
                            x_res[:, kk, j * 512:(j + 1) * 512],
                            start=(kk == 0), stop=False,
                        )
                agi = ag[I_OF_MB[mb]]
                for j in range(NJ):
                    nc.tensor.matmul(
                        pss[j][:],
                        b_res[:, mb, :],
                        agi[:, j * 512:(j + 1) * 512],
                        start=False, stop=True,
                    )
                st = st_pool.tile([P, TC], bf16, tag="st", name=f"st{mb}")
                for j in range(NJ):
                    nc.vector.tensor_copy(st[:, j * 512:(j + 1) * 512], pss[j][:])
                nc.scalar.dma_start(out=y_d[mb], in_=st[:])

    nc.compile()
    return nc


def _get_nc():
    if "nc" not in _CACHE:
        _CACHE["nc"] = _build_nc()
    return _CACHE["nc"]


def _prep_in_maps(x, W, lora_A, lora_B_q, lora_B_k, lora_B_v, scaling, token_to_slot):
    f = np.float32
    # moving operand per token shard: [tq][p(k), kt, t]
    x_sh = np.ascontiguousarray(
        np.asarray(x).astype(BF16).reshape(TS, TC, KA, P).transpose(0, 3, 2, 1))
    # W stationary per output shard: [dh][mb, p(k), kt, m]
    Wb = np.asarray(W).astype(BF16)
    w_sh = [
        np.ascontiguousarray(
            Wb[_rows_of_dh(dh)].reshape(MB, P, KA, P).transpose(0, 3, 2, 1))
        for dh in range(DS)
    ]
    # LoRA A stationary: [p(k), kt, i, (s r)]
    a_sh = np.ascontiguousarray(
        np.asarray(lora_A).astype(BF16).reshape(S, 3, R, KA, P)
        .transpose(4, 3, 1, 0, 2).reshape(P, KA, 3, S * R))
    # LoRA B stationary: [(s r), mb, m] per output shard
    bq = np.asarray(lora_B_q).astype(BF16)
    bk = np.asarray(lora_B_k).astype(BF16)
    bv = np.asarray(lora_B_v).astype(BF16)
    b_full = np.concatenate([bq, bk, bv], axis=1).transpose(0, 2, 1).reshape(S * R, D)
    b_sh = [
        np.ascontiguousarray(b_full[:, _rows_of_dh(dh)].reshape(S * R, MB, P))
        for dh in range(DS)
    ]
    # routing gate, expanded over ranks: [tq][(s r), t]  (fp32, exact)
    slot = np.asarray(token_to_slot).reshape(TS, TC)
    g = (slot[:, None, :] == np.arange(S, dtype=slot.dtype)[None, :, None])
    g = g.astype(f) * np.asarray(scaling, dtype=f)[None, :, None]
    gate = np.ascontiguousarray(np.repeat(g, R, axis=1))

    in_maps = []
    for c in range(NCORES):
        dh, tq = c // TS, c % TS
        in_maps.append({
            "x_sh": x_sh[tq],
            "w_sh": w_sh[dh],
            "a_sh": a_sh,
            "b_sh": b_sh[dh],
            "gate": gate[tq],
        })
    return in_maps


def _assemble(results):
    out = np.empty((T, D), dtype=np.float32)
    for c in range(NCORES):
        dh, tq = c // TS, c % TS
        blk = np.asarray(results[c]["y_out"]).reshape(DH, TC)
        out[tq * TC:(tq + 1) * TC, _rows_of_dh(dh)] = blk.T
    return out


def _run(inputs, trace=False):
    from concourse.bass_utils import run_bass_kernel_spmd
    nc = _get_nc()
    in_maps = _prep_in_maps(**inputs)
    res = run_bass_kernel_spmd(
        nc, in_maps, core_ids=list(range(NCORES)), trace=trace)
    return res


def kernel(**inputs) -> np.ndarray:
    res = _run(inputs, trace=False)
    return _assemble(res.results)


if __name__ == "__main__":
    rng = np.random.default_rng(0)
    ins = {
        "x": rng.standard_normal((T, HID)).astype(np.float32),
        "W": (rng.standard_normal((D, HID)) * 0.02).astype(np.float32),
        "lora_A": (rng.standard_normal((S, 3, R, HID)) * 0.02).astype(np.float32),
        "lora_B_q": (rng.standard_normal((S, Q_SIZE, R)) * 0.02).astype(np.float32),
        "lora_B_k": (rng.standard_normal((S, KV_SIZE, R)) * 0.02).astype(np.float32),
        "lora_B_v": (rng.standard_normal((S, KV_SIZE, R)) * 0.02).astype(np.float32),
        "scaling": rng.uniform(0.5, 2.0, S).astype(np.float32),
        "token_to_slot": rng.integers(0, S, T).astype(np.int32),
    }
    out = kernel(**ins)
    print("out", out.shape, out.dtype)


# revision 3
# speedup vs baseline: 1.2787x; 1.1217x over previous
"""LoRA QKV fused projection kernel for 8 TRN2 NeuronCores.

Reference computation (T=8192 tokens, HID=4096, D=6144 out, S=8 slots, R=16):
    y = x @ W.T
    a[t,s,i,r] = sum_h x[t,h] * lora_A[s,i,r,h]          (down-proj, all slots)
    a *= onehot(token_to_slot)[t,s] * scaling[s]          (routing gate)
    d[t, :] = concat_i( sum_{s,r} a[t,s,i,r] * B_i[s,:,r] )   (up-proj)
    out = y + d

Sharding: pure 8-way token split (core c owns tokens [c*1024,(c+1)*1024),
full output dim).  This makes the per-core PE work exactly total/8 — no
LoRA down-proj duplication — at the cost of streaming the full W
(50 MB bf16) per core, which hides comfortably under the ~700 us of
matmul.

All matmul operands are bf16 (fp32 PSUM accumulate; PE runs 1
column/cycle at 128x128, same rate as fp32r, but bf16 halves every DMA
and SBUF footprint).  Per core:
  * Phase A (LoRA down-proj): aT[i] = A_i @ x_shard^T accumulated in 6
    PSUM banks over 32 k-tiles (k outermost so it streams behind the
    chunked x DMA), gated (onehot*scaling, host-built fp32) into bf16
    ag tiles.
  * Main loop over 48 row-blocks of 128: y-tile accumulated over 32
    k-tiles with W stationary, then ONE extra matmul accumulates the
    LoRA delta (B stationary, ag moving) into the same PSUM bank before
    the fp32->bf16 store.
Host: pure layout rearranges; final assembly is one cast+transpose per
block (no reduce, no permutation).
"""

import numpy as np
import ml_dtypes

# problem shape (hardcoded per harness contract)
T = 8192
HID = 4096
Q_SIZE = 4096
KV_SIZE = 1024
D = Q_SIZE + 2 * KV_SIZE  # 6144
S = 8
R = 16
NCORES = 8
P = 128

TC = T // NCORES          # 1024 tokens per core
MB = D // P               # 48 output row-blocks of 128
KA = HID // P             # 32 k-tiles
NJ = TC // 512            # 2 moving n-tiles of 512 tokens
KCH = 8                   # k-tiles per streamed input chunk
I_OF_MB = [0] * (Q_SIZE // P) + [1] * (KV_SIZE // P) + [2] * (KV_SIZE // P)

BF16 = np.dtype(ml_dtypes.bfloat16)

_CACHE = {}


def _build_nc():
    import concourse.mybir as mybir
    import concourse.tile as tile
    from concourse import bacc

    bf16 = mybir.dt.bfloat16
    f32 = mybir.dt.float32

    nc = bacc.Bacc(None, target_bir_lowering=False, debug=False)

    x_d = nc.declare_dram_parameter("x_sh", [P, KA, TC], bf16, isOutput=False)
    w_d = nc.declare_dram_parameter("w_sh", [MB, P, KA, P], bf16, isOutput=False)
    a_d = nc.declare_dram_parameter("a_sh", [P, KA, 3, P], bf16, isOutput=False)
    b_d = nc.declare_dram_parameter("b_sh", [P, MB, P], bf16, isOutput=False)
    g_d = nc.declare_dram_parameter("gate", [P, TC], f32, isOutput=False)
    y_d = nc.declare_dram_parameter("y_out", [MB, P, TC], bf16, isOutput=True)

    with tile.TileContext(nc) as tc:
        with tc.tile_pool(name="xres", bufs=1) as xres_pool, \
             tc.tile_pool(name="wp", bufs=3) as w_pool, \
             tc.tile_pool(name="cst", bufs=1) as cst_pool, \
             tc.tile_pool(name="agp", bufs=1) as ag_pool, \
             tc.tile_pool(name="stp", bufs=3) as st_pool, \
             tc.tile_pool(name="psum", bufs=8, space="PSUM") as ps_pool:

            # resident moving operand: [p(k), kt, t]; streamed in k-chunks
            # (interleaved with the A chunks phase A needs at the same kk)
            x_res = xres_pool.tile([P, KA, TC], bf16, tag="xres")
            a_res = cst_pool.tile([P, KA, 3, P], bf16, tag="a")
            for ch in range(KA // KCH):
                sl = slice(ch * KCH, (ch + 1) * KCH)
                nc.sync.dma_start(out=a_res[:, sl], in_=a_d[:, sl])
                nc.sync.dma_start(out=x_res[:, sl], in_=x_d[:, sl])
            gate_t = cst_pool.tile([P, TC], f32, tag="gate")
            nc.sync.dma_start(out=gate_t[:], in_=g_d[:])
            b_res = cst_pool.tile([P, MB, P], bf16, tag="b")
            nc.sync.dma_start(out=b_res[:], in_=b_d[:])

            # ---------------- Phase A: LoRA down-proj + gate ----------------
            ps_a = [
                ps_pool.tile([P, 512], f32, tag="ps", name=f"pa{i}_{j}")
                for i in range(3) for j in range(NJ)
            ]
            for kk in range(KA):
                for i in range(3):
                    for j in range(NJ):
                        nc.tensor.matmul(
                            ps_a[i * NJ + j][:],
                            a_res[:, kk, i, :],
                            x_res[:, kk, j * 512:(j + 1) * 512],
                            start=(kk == 0), stop=(kk == KA - 1),
                        )
            ag = []
            for i in range(3):
                ag_t = ag_pool.tile([P, TC], bf16, tag=f"ag{i}", name=f"ag{i}")
                for j in range(NJ):
                    sl = slice(j * 512, (j + 1) * 512)
                    nc.vector.tensor_mul(ag_t[:, sl], ps_a[i * NJ + j][:], gate_t[:, sl])
                ag.append(ag_t)

            # ------------- Main GEMM + fused LoRA up-proj -------------------
            for mb in range(MB):
                w_t = w_pool.tile([P, KA, P], bf16, tag="w", name=f"w{mb}")
                nc.sync.dma_start(out=w_t[:], in_=w_d[mb])
                pss = [
                    ps_pool.tile([P, 512], f32, tag="ps", name=f"pm{mb}_{j}")
                    for j in range(NJ)
                ]
                for kk in range(KA):
                    for j in range(NJ):
                        nc.tensor.matmul(
                            pss[j][:],
                            w_t[:, kk, :],
                            x_res[:, kk, j * 512:(j + 1) * 512],
                            start=(kk == 0), stop=False,
                        )
                agi = ag[I_OF_MB[mb]]
                for j in range(NJ):
                    nc.tensor.matmul(
                        pss[j][:],
                        b_res[:, mb, :],
                        agi[:, j * 512:(j + 1) * 512],
                        start=False, stop=True,
                    )
                st = st_pool.tile([P, TC], bf16, tag="st", name=f"st{mb}")
                for j in range(NJ):
                    nc.vector.tensor_copy(st[:, j * 512:(j + 1) * 512], pss[j][:])
                nc.scalar.dma_start(out=y_d[mb], in_=st[:])

    nc.compile()
    return nc


def _get_nc():
    if "nc" not in _CACHE:
        _CACHE["nc"] = _build_nc()
    return _CACHE["nc"]


def _prep_in_maps(x, W, lora_A, lora_B_q, lora_B_k, lora_B_v, scaling, token_to_slot):
    f = np.float32
    # moving operand per token shard: [c][p(k), kt, t]
    x_sh = np.ascontiguousarray(
        np.asarray(x).astype(BF16).reshape(NCORES, TC, KA, P).transpose(0, 3, 2, 1))
    # W stationary (shared by all cores): [mb, p(k), kt, m]
    w_sh = np.ascontiguousarray(
        np.asarray(W).astype(BF16).reshape(MB, P, KA, P).transpose(0, 3, 2, 1))
    # LoRA A stationary: [p(k), kt, i, (s r)]
    a_sh = np.ascontiguousarray(
        np.asarray(lora_A).astype(BF16).reshape(S, 3, R, KA, P)
        .transpose(4, 3, 1, 0, 2).reshape(P, KA, 3, S * R))
    # LoRA B stationary (shared): [(s r), mb, m]
    bq = np.asarray(lora_B_q).astype(BF16)
    bk = np.asarray(lora_B_k).astype(BF16)
    bv = np.asarray(lora_B_v).astype(BF16)
    b_sh = np.ascontiguousarray(
        np.concatenate([bq, bk, bv], axis=1).transpose(0, 2, 1)
        .reshape(S * R, MB, P))
    # routing gate, expanded over ranks: [c][(s r), t]  (fp32, exact)
    slot = np.asarray(token_to_slot).reshape(NCORES, TC)
    g = (slot[:, None, :] == np.arange(S, dtype=slot.dtype)[None, :, None])
    g = g.astype(f) * np.asarray(scaling, dtype=f)[None, :, None]
    gate = np.ascontiguousarray(np.repeat(g, R, axis=1))

    in_maps = []
    for c in range(NCORES):
        in_maps.append({
            "x_sh": x_sh[c],
            "w_sh": w_sh,
            "a_sh": a_sh,
            "b_sh": b_sh,
            "gate": gate[c],
        })
    return in_maps


def _assemble(results):
    out = np.empty((T, D), dtype=np.float32)
    for c in range(NCORES):
        blk = np.asarray(results[c]["y_out"]).reshape(D, TC)
        out[c * TC:(c + 1) * TC, :] = blk.T
    return out


def _run(inputs, trace=False):
    from concourse.bass_utils import run_bass_kernel_spmd
    nc = _get_nc()
    in_maps = _prep_in_maps(**inputs)
    res = run_bass_kernel_spmd(
        nc, in_maps, core_ids=list(range(NCORES)), trace=trace)
    return res


def kernel(**inputs) -> np.ndarray:
    res = _run(inputs, trace=False)
    return _assemble(res.results)


if __name__ == "__main__":
    rng = np.random.default_rng(0)
    ins = {
        "x": rng.standard_normal((T, HID)).astype(np.float32),
        "W": (rng.standard_normal((D, HID)) * 0.02).astype(np.float32),
        "lora_A": (rng.standard_normal((S, 3, R, HID)) * 0.02).astype(np.float32),
        "lora_B_q": (rng.standard_normal((S, Q_SIZE, R)) * 0.02).astype(np.float32),
        "lora_B_k": (rng.standard_normal((S, KV_SIZE, R)) * 0.02).astype(np.float32),
        "lora_B_v": (rng.standard_normal((S, KV_SIZE, R)) * 0.02).astype(np.float32),
        "scaling": rng.uniform(0.5, 2.0, S).astype(np.float32),
        "token_to_slot": rng.integers(0, S, T).astype(np.int32),
    }
    out = kernel(**ins)
    print("out", out.shape, out.dtype)
